# revision 1
# baseline (speedup 1.0000x reference)
"""Trainium2 Bass kernel for nn_MultiHeadAttention_47485158424810.

Full-input contract: kernel(**inputs) takes the unsharded numpy inputs and
returns the full [2, 2048, 1024] output.

Sharding (8 cores): core = b*4 + hg
  - data parallel over batch b in {0,1}
  - tensor parallel over 4 head-groups hg (4 heads of 64 dims each -> 256
    output dims per core) by splitting Wq/Wk/Wv rows (column-parallel) and
    Wo columns (row-parallel).  Each core emits a partial [2048, 1024]
    output; the host sums the 4 partials per batch and adds Wo_b.

Device-side plan per core (T=2048, K=1024, O=256, 4 heads of s=64), all
matmuls in float32r (full PE rate at moving dim >= 256):
  phase 1: x arrives host-pre-transposed ([K, T]); stream per 512-token
           chunk; project Q^T,K^T ([256, 2048], head pairs packed at
           partition offsets 0/64) and V (natural [t, o] layout, packed
           per t-tile with a ones column per head for softmax row sums).
  phase 2: per (head pair, q-chunk of 512): S^T tiles for both heads go in
           one [128, 1024] psum tile via row-group-packed matmuls; exp on
           ACT (scale=1/sqrt(K) folded in; ACT runs ONLY exp so its table
           never swaps); causal masking via precomputed bf16 0/1 masks on
           DVE with fully-masked columns skipped end-to-end; attn@V on PE
           with the ones column accumulating softmax denominators free.
  phase 3: normalize O^T by PE-broadcast reciprocal row sums (ones x recip
           rank-1 matmul), add V-bias on DVE (exact: softmax rows sum to
           1), Wo row-parallel matmul, DMA the partial output.
"""

import os
import sys

import numpy as np

for _p in ("/root/.axon_site/_ro/trn_rl_repo", "/opt/trn_rl_repo"):
    if os.path.isdir(_p) and _p not in sys.path:
        sys.path.append(_p)

import concourse.bass as bass
import concourse.tile as tile
from concourse import bacc, mybir
from concourse.bass_utils import run_bass_kernel_spmd

B, T, K, H = 2, 2048, 1024, 16
NCORES = 8
O = 256  # head-group width per core (4 heads x 64)
S = 64  # head dim
HPC = 4  # heads per core
F32 = mybir.dt.float32
F32R = mybir.dt.float32r
AF = mybir.ActivationFunctionType
ALU = mybir.AluOpType

_CACHE = {}


def _build_body(nc, tc, d, loop_n=0):
    if loop_n:
        with tc.For_i(0, loop_n, 1):
            with tc.tile_pool(name="consts", bufs=1) as consts, \
                 tc.tile_pool(name="persist", bufs=1) as persist, \
                 tc.tile_pool(name="pss", bufs=2, space="PSUM") as pss_p:
                _build_inner(nc, tc, d, consts, persist, pss_p)
        return
    with tc.tile_pool(name="consts", bufs=1) as consts, \
         tc.tile_pool(name="persist", bufs=1) as persist, \
         tc.tile_pool(name="pss", bufs=2, space="PSUM") as pss_p:
        _build_inner(nc, tc, d, consts, persist, pss_p)


def _build_inner(nc, tc, d, consts, persist, pss_p):
    f32 = F32
    x_d, wq_d, wk_d, wv_d, wo_d, bq_d, bk_d, bv_d, y_d = (
        d["x"], d["wqT"], d["wkT"], d["wvT"], d["woT"],
        d["bq"], d["bk"], d["bv"], d["y"],
    )
    def load_wT(ap_d, prefix, eng):
        tiles = []
        for kk in range(8):
            t_ = consts.tile([128, O], F32R, name=f"{prefix}{kk}")
            eng.dma_start(t_, ap_d[kk * 128:(kk + 1) * 128, :])
            tiles.append(t_)
        return tiles

    # spread weight loads over idle queues so the first projections and
    # exps are not stuck behind 24 serial dispatches on one queue
    wq_sb = load_wT(wq_d, "wq", nc.scalar)
    wk_sb = load_wT(wk_d, "wk", nc.scalar)
    wv_sb = load_wT(wv_d, "wv", nc.gpsimd)
    wo_sb = []
    for oc in range(2):
        t_ = consts.tile([128, K], F32R, name=f"wo{oc}")
        nc.gpsimd.dma_start(t_, wo_d[oc * 128:(oc + 1) * 128, :])
        wo_sb.append(t_)

    def load_bias(ap_d, nm):
        t_ = consts.tile([128, 2], f32, name=nm)
        nc.gpsimd.dma_start(t_, ap_d.rearrange("(c p) -> p c", p=128))
        return t_

    # After column trimming, the partially-masked region of a diagonal
    # tile is always the first 128 columns of its trimmed view, with the
    # same triangular predicate (keep i <= j') for every m.  One [128,
    # 2x128] bf16 mask (twin halves for the head pair) covers all cases.
    trimask = consts.tile([128, 256], mybir.dt.bfloat16, name="trimask")
    nc.gpsimd.memset(trimask, 1.0)
    tm3 = trimask.rearrange("p (e j) -> p e j", e=2)
    nc.gpsimd.affine_select(
        out=tm3, in_=tm3, pattern=[[0, 2], [1, 128]],
        compare_op=ALU.is_ge, fill=0.0, base=0, channel_multiplier=-1)

    bq_sb = load_bias(bq_d, "bq_sb")
    bk_sb = load_bias(bk_d, "bk_sb")
    bv_sb = load_bias(bv_d, "bv_sb")

    # persistent activations
    qT = [persist.tile([128, T], F32R, name=f"qT{oc}") for oc in range(2)]
    kT = [persist.tile([128, T], F32R, name=f"kT{oc}") for oc in range(2)]
    oT = [persist.tile([128, T], F32R, name=f"oT{oc}") for oc in range(2)]
    # V natural layout, per t_k tile: 4 heads x (64 dims + ones col)
    vv = [persist.tile([128, HPC * (S + 1)], F32R, name=f"v{i}")
          for i in range(T // 128)]
    # softmax denominators, one [1, T] tile per head (base partition 0)
    rsum = [persist.tile([1, T], F32R, name=f"rsum{h}") for h in range(HPC)]
    ones128 = persist.tile([128, 128], F32R, name="ones128")
    ones_f32 = persist.tile([128, 128], f32, name="ones_f32")
    nc.gpsimd.memset(ones_f32, 1.0)
    # f32r tiles can't be memset directly; DVE copy rounds f32 -> f32r
    nc.vector.tensor_copy(ones128, ones_f32)

    for i in range(T // 128):
        # ones column at offset h*(S+1)+S for each head
        nc.vector.tensor_copy(vv[i][:, S::S + 1], ones_f32[:, 0:HPC])

    # ------- interleaved projections + attention, per 512-token chunk -------
    # Phase-1 is PE-heavy with ACT idle; the attention inner loop saturates
    # ACT (exp) with PE slack.  Emitting proj(c) then attention(c) per chunk
    # with coexisting psum pools lets the Tile scheduler overlap them.
    inv_scale = 1.0 / float(np.sqrt(K))
    with tc.tile_pool(name="xTp", bufs=2) as xT_p, \
         tc.tile_pool(name="ppr", bufs=2, space="PSUM") as ppr_p, \
         tc.tile_pool(name="pso", bufs=1, space="PSUM") as pso_p, \
         tc.tile_pool(name="ptile", bufs=3) as pt_p:
        for c in range(4):  # chunks of 512 tokens
            tch = c
            xT = [
                xT_p.tile([128, 512], F32R, name=f"xT{tch}_{kk}", tag=f"xT{kk}")
                for kk in range(8)
            ]
            for kk in range(8):
                eng = nc.sync if kk % 2 == 0 else nc.scalar
                eng.dma_start(
                    xT[kk],
                    x_d[kk * 128:(kk + 1) * 128,
                        tch * 512:(tch + 1) * 512])
            # Q^T and K^T: [o on partitions, t free]
            for w_sb, b_sb, dest in ((wq_sb, bq_sb, qT), (wk_sb, bk_sb, kT)):
                for oc in range(2):
                    ps = ppr_p.tile([128, 512], f32, name="ps_qk", tag="ps")
                    for kk in range(8):
                        nc.tensor.matmul(
                            ps,
                            w_sb[kk][:, oc * 128:(oc + 1) * 128],
                            xT[kk],
                            start=(kk == 0), stop=(kk == 7))
                    nc.vector.tensor_scalar_add(
                        dest[oc][:, tch * 512:(tch + 1) * 512], ps,
                        b_sb[:, oc:oc + 1])
            # V natural: [t on partitions, o free]; no bias (folded later)
            for a in range(4):
                ps = ppr_p.tile([128, O], f32, name="ps_v", tag="ps")
                for kk in range(8):
                    nc.tensor.matmul(
                        ps,
                        xT[kk][:, a * 128:(a + 1) * 128],
                        wv_sb[kk],
                        start=(kk == 0), stop=(kk == 7))
                for h in range(HPC):
                    nc.vector.tensor_copy(
                        vv[tch * 4 + a][:, h * (S + 1):h * (S + 1) + S],
                        ps[:, h * S:(h + 1) * S])

            # ---- attention for q-chunk c ----
            for oc in range(2):  # head pair (2*oc, 2*oc+1)
                po = [pso_p.tile([S + 1, 512], f32, name=f"po{e}",
                                 tag=f"po{e}") for e in range(2)]
                nr = 4 * (c + 1)  # causal: t_k tiles 0..4c+3
                for r in range(nr):
                    m = r - 4 * c
                    j0 = 128 * m if m > 0 else 0  # fully-masked cols skipped
                    # one [128, 1024] psum tile holds S^T for BOTH heads of
                    # the pair; their matmuls use disjoint 64-partition row
                    # groups so the PE overlaps them.
                    ps = pss_p.tile([128, 1024], f32, name="ps_s", tag="pss")
                    for e in range(2):
                        hb = e * 64
                        nc.tensor.matmul(
                            ps[:, e * 512 + j0:(e + 1) * 512],
                            kT[oc][hb:hb + 64, r * 128:(r + 1) * 128],
                            qT[oc][hb:hb + 64, c * 512 + j0:(c + 1) * 512],
                            start=True, stop=True)
                    pt = pt_p.tile([128, 1024], F32R, name="pt_exp",
                                   tag="ptl", bufs=6)
                    ps3 = ps.rearrange("p (e j) -> p e j", e=2)[:, :, j0:]
                    pt3 = pt.rearrange("p (e j) -> p e j", e=2)[:, :, j0:]
                    # exp over both heads in one ACT op
                    nc.scalar.activation(pt3, ps3, AF.Exp, scale=inv_scale)
                    if m >= 0:
                        # only the first 128 trimmed columns are partial
                        nc.vector.tensor_mul(
                            pt3[:, :, 0:128], pt3[:, :, 0:128],
                            trimask.rearrange("p (e j) -> p e j", e=2))
                    for e in range(2):
                        h = 2 * oc + e
                        nc.tensor.matmul(
                            po[e][:, j0:],
                            vv[r][:, h * (S + 1):(h + 1) * (S + 1)],
                            pt[:, e * 512 + j0:(e + 1) * 512],
                            start=(r == 0), stop=(r == nr - 1))
                # evict O^T rows (unnormalized) + denominator row
                for e in range(2):
                    h = 2 * oc + e
                    hb = e * 64
                    stg = pt_p.tile([S + 1, 512], F32R, name=f"ostg{e}",
                                    tag=f"ostg{e}", bufs=2)
                    nc.vector.tensor_copy(stg, po[e])
                    nc.gpsimd.dma_start(
                        oT[oc][hb:hb + 64, c * 512:(c + 1) * 512], stg[0:S, :])
                    nc.gpsimd.dma_start(
                        rsum[h][0:1, c * 512:(c + 1) * 512], stg[S:S + 1, :])

    # ---------------- phase 3: normalize + output projection ----------------
    with tc.tile_pool(name="ystg", bufs=4) as ystg_p, \
         tc.tile_pool(name="psrb", bufs=2, space="PSUM") as psrb_p, \
         tc.tile_pool(name="psy", bufs=2, space="PSUM") as psy_p:
        with nc.allow_low_precision(reason="f32r reciprocal of softmax denom"):
            for h in range(HPC):
                nc.vector.reciprocal(rsum[h], rsum[h])
        for h in range(HPC):
            hb = (h % 2) * 64
            oc = h // 2
            for c in range(4):
                # broadcast recip row across partitions: ones[:,0:1] x recip
                prb = psrb_p.tile([128, 512], f32, name="prb", tag="prb")
                nc.tensor.matmul(
                    prb, ones128[0:1, :],
                    rsum[h][0:1, c * 512:(c + 1) * 512],
                    start=True, stop=True)
                nc.vector.tensor_mul(
                    oT[oc][hb:hb + 64, c * 512:(c + 1) * 512],
                    oT[oc][hb:hb + 64, c * 512:(c + 1) * 512],
                    prb[hb:hb + 64, :])
        for c in range(4):
            for oc in range(2):
                # + V bias: exact since softmax rows sum to 1 (DVE keeps
                # the ACT exp table resident; per-chunk ops unblock the
                # output projection early)
                nc.vector.tensor_scalar_add(
                    oT[oc][:, c * 512:(c + 1) * 512],
                    oT[oc][:, c * 512:(c + 1) * 512],
                    bv_sb[:, oc:oc + 1])
        for i in range(T // 128):
            ys = ystg_p.tile([128, K], f32, name="ystg", tag="ystg")
            for jc in range(2):
                py = psy_p.tile([128, 512], f32, name="py", tag="py")
                for oc in range(2):
                    nc.tensor.matmul(
                        py,
                        oT[oc][:, i * 128:(i + 1) * 128],
                        wo_sb[oc][:, jc * 512:(jc + 1) * 512],
                        start=(oc == 0), stop=(oc == 1))
                # DVE saturates in the output tail while ACT idles (exp is
                # finished - one table swap); split the psum evictions
                if (2 * i + jc) % 2 == 0:
                    nc.vector.tensor_copy(ys[:, jc * 512:(jc + 1) * 512], py)
                else:
                    nc.scalar.copy(ys[:, jc * 512:(jc + 1) * 512], py)
            # drain the 8 MB output on two DMA queues
            eng = nc.sync if i % 2 == 0 else nc.gpsimd
            eng.dma_start(y_d[i * 128:(i + 1) * 128, :], ys)


def build_program(loop_n=0):
    nc = bacc.Bacc("TRN2", target_bir_lowering=False, debug=False,
                   num_devices=NCORES)
    d = {
        "x": nc.dram_tensor("xT", [K, T], F32R, kind="ExternalInput").ap(),
        "wqT": nc.dram_tensor("wqT", [K, O], F32R, kind="ExternalInput").ap(),
        "wkT": nc.dram_tensor("wkT", [K, O], F32R, kind="ExternalInput").ap(),
        "wvT": nc.dram_tensor("wvT", [K, O], F32R, kind="ExternalInput").ap(),
        "woT": nc.dram_tensor("woT", [O, K], F32R, kind="ExternalInput").ap(),
        "bq": nc.dram_tensor("bq", [O], F32, kind="ExternalInput").ap(),
        "bk": nc.dram_tensor("bk", [O], F32, kind="ExternalInput").ap(),
        "bv": nc.dram_tensor("bv", [O], F32, kind="ExternalInput").ap(),
        "y": nc.dram_tensor("y", [T, K], F32, kind="ExternalOutput").ap(),
    }
    with tile.TileContext(nc) as tc:
        _build_body(nc, tc, d, loop_n=loop_n)
    nc.compile()
    return nc


def _get_program():
    if "nc" not in _CACHE:
        _CACHE["nc"] = build_program()
    return _CACHE["nc"]


def make_in_maps(x, Wq_w, Wk_w, Wv_w, Wo_w, Wq_b, Wk_b, Wv_b):
    in_maps = []
    for core in range(NCORES):
        b, hg = divmod(core, 4)
        sl = slice(hg * O, (hg + 1) * O)
        in_maps.append({
            "xT": np.ascontiguousarray(x[b].T, np.float32),
            "wqT": np.ascontiguousarray(Wq_w[sl, :].T, np.float32),
            "wkT": np.ascontiguousarray(Wk_w[sl, :].T, np.float32),
            "wvT": np.ascontiguousarray(Wv_w[sl, :].T, np.float32),
            "woT": np.ascontiguousarray(Wo_w[:, sl].T, np.float32),
            "bq": np.ascontiguousarray(Wq_b[sl], np.float32),
            "bk": np.ascontiguousarray(Wk_b[sl], np.float32),
            "bv": np.ascontiguousarray(Wv_b[sl], np.float32),
        })
    return in_maps


def _combine(results, Wo_b):
    y = np.empty((B, T, K), np.float32)
    for b in range(B):
        acc = results[b * 4]["y"].copy()
        for hg in range(1, 4):
            acc += results[b * 4 + hg]["y"]
        y[b] = acc + np.asarray(Wo_b, np.float32)
    return y


def kernel(x, Wq_w, Wq_b, Wk_w, Wk_b, Wv_w, Wv_b, Wo_w, Wo_b):
    x = np.asarray(x, np.float32)
    nc = _get_program()
    in_maps = make_in_maps(x, np.asarray(Wq_w), np.asarray(Wk_w),
                           np.asarray(Wv_w), np.asarray(Wo_w),
                           np.asarray(Wq_b), np.asarray(Wk_b),
                           np.asarray(Wv_b))
    out = run_bass_kernel_spmd(nc, in_maps, list(range(NCORES)))
    return _combine(out.results, Wo_b)



# revision 24
# speedup vs baseline: 6.0639x; 6.0639x over previous
"""Trainium2 Bass kernel for nn_MultiHeadAttention_47485158424810.

Full-input contract: kernel(**inputs) takes the unsharded numpy inputs and
returns the full [2, 2048, 1024] output.

Sharding (8 cores): core = b*4 + hg
  - data parallel over batch b in {0,1}
  - tensor parallel over 4 head-groups hg (4 heads of 64 dims each -> 256
    output dims per core) by splitting Wq/Wk/Wv rows (column-parallel) and
    Wo columns (row-parallel).  Each core emits a partial [2048, 1024]
    output; the host sums the 4 partials per batch and adds the combined
    bias vector Wo_w @ Wv_b + Wo_b (softmax rows sum to 1, so the V bias
    contributes a constant; the K bias cancels exactly in softmax and is
    dropped).

Device-side plan per core (T=2048, K=1024, O=256, 4 heads of s=64), bf16
matmul inputs with f32 PSUM accumulation:
  Fully fused per-512-token-chunk pipeline, ordered so the list scheduler
  always has dense PE work (keeps the HAM clock gate at 2.4 GHz):
    prologue: xT(0) + weight DMAs spread over idle queues; proj(0)
    chunk c:  emit proj(c+1) first (PE filler during ACT-bound attention),
              then attention(c): per (head pair, r): S^T for both heads in
              one [128,1024] psum via row-group-packed matmuls; exp on ACT
              (scale 1/sqrt(K) folded; Copy shares the exp table so no
              swaps); causal masking via a bf16 0/1 mask on the idle Pool
              engine; attn@V with a ones column accumulating softmax
              denominators for free;
              then per-chunk epilogue: evict O^T (head 0 direct DVE copy,
              head 1 via bf16 staging + partition-shift DMA), reciprocal
              of the denominators straight out of PSUM, PE rank-1
              broadcast of the reciprocals, normalize, and the row-parallel
              Wo projection + streaming y DMA.
  PSUM plan (8 banks): proj ring 2, S^T/outproj ring 2x[128,1024]=4,
  po0(+prb) 1, po1 1.
"""

import os
import sys

import numpy as np

for _p in ("/root/.axon_site/_ro/trn_rl_repo", "/opt/trn_rl_repo"):
    if os.path.isdir(_p) and _p not in sys.path:
        sys.path.append(_p)

import concourse.bass as bass
import concourse.tile as tile
from concourse import bacc, mybir
from concourse.bass_utils import run_bass_kernel_spmd

B, T, K, H = 2, 2048, 1024, 16
NCORES = 8
O = 256  # head-group width per core (4 heads x 64)
S = 64  # head dim
HPC = 4  # heads per core
F32 = mybir.dt.float32
F32R = mybir.dt.float32r
BF16 = mybir.dt.bfloat16
AF = mybir.ActivationFunctionType
ALU = mybir.AluOpType

_CACHE = {}


def _build_body(nc, tc, d, loop_n=0):
    if loop_n:
        with tc.For_i(0, loop_n, 1):
            with tc.tile_pool(name="consts", bufs=1) as consts, \
                 tc.tile_pool(name="persist", bufs=1) as persist, \
                 tc.tile_pool(name="work", bufs=1) as work, \
                 tc.tile_pool(name="pmm", bufs=2, space="PSUM") as pmm, \
                 tc.tile_pool(name="pbig", bufs=2, space="PSUM") as pbig, \
                 tc.tile_pool(name="ppo", bufs=1, space="PSUM") as ppo:
                _build_inner(nc, tc, d, consts, persist, work, pmm, pbig, ppo)
        return
    with tc.tile_pool(name="consts", bufs=1) as consts, \
         tc.tile_pool(name="persist", bufs=1) as persist, \
         tc.tile_pool(name="work", bufs=1) as work, \
         tc.tile_pool(name="pmm", bufs=2, space="PSUM") as pmm, \
         tc.tile_pool(name="pbig", bufs=2, space="PSUM") as pbig, \
         tc.tile_pool(name="ppo", bufs=1, space="PSUM") as ppo:
        _build_inner(nc, tc, d, consts, persist, work, pmm, pbig, ppo)


def _build_inner(nc, tc, d, consts, persist, work, pmm, pbig, ppo):
    f32 = F32
    x_d, wq_d, wk_d, wv_d, wo_d, bq_d, y_d = (
        d["x"], d["wqT"], d["wkT"], d["wvT"], d["woT"], d["bq"], d["y"],
    )

    # ---------------- persistent tiles ----------------
    # per-chunk q/k tiles (NOT one [128, T] tile: whole-tile RAW tracking
    # would make attention(c) reads wait on proj(c+1) writes)
    qT = [[persist.tile([128, 512], BF16, name=f"qT{c}_{oc}")
           for oc in range(2)] for c in range(4)]
    kT = [[persist.tile([128, 512], BF16, name=f"kT{c}_{oc}")
           for oc in range(2)] for c in range(4)]
    # V natural layout, per t_k tile: 4 heads x (64 dims + ones col)
    vv = [persist.tile([128, HPC * (S + 1)], BF16, name=f"v{i}")
          for i in range(T // 128)]
    ones_f32 = persist.tile([128, 128], f32, name="ones_f32")
    onesr = persist.tile([128, 128], F32R, name="onesr")

    # ---------------- masks / ones / PE warmup (cheap Pool engine ops;
    # emitted first so their dispatches precede the SWDGE descriptor
    # generation, which costs ~1us of Pool sequencer time per DMA) -------
    trimask = consts.tile([128, 256], BF16, name="trimask")
    nc.gpsimd.memset(trimask, 1.0)
    tm3 = trimask.rearrange("p (e j) -> p e j", e=2)
    nc.gpsimd.affine_select(
        out=tm3, in_=tm3, pattern=[[0, 2], [1, 128]],
        compare_op=ALU.is_ge, fill=0.0, base=0, channel_multiplier=-1)
    nc.gpsimd.memset(ones_f32, 1.0)
    nc.vector.tensor_copy(onesr, ones_f32)  # f32r via DVE round
    for i in range(T // 128):
        # ones column at offset h*65+64 for each head
        nc.vector.tensor_copy(vv[i][:, S::S + 1], ones_f32[:, 0:HPC])

    # warm the PE (HAM clock gate) while the first DMAs land
    warm = pbig.tile([128, 1024], f32, name="warm", tag="pss")
    for w in range(14):
        nc.tensor.matmul(warm[:, 0:128], onesr, onesr,
                         start=(w == 0), stop=(w == 13))

    # ---------------- DMA loads, spread over idle queues ----------------
    # Weights are batched into ONE strided DMA per matrix half (per-tile
    # DMAs trickle in at ~1us each and gate the first projection).  x is
    # host-pre-tiled chunk-major so each chunk load is 128 contiguous
    # descriptors.
    xT_tiles = {}

    def emit_x_chunk(c):
        t_ = work.tile([128, 8 * 512], BF16, name=f"xT{c}", tag="xT", bufs=2)
        nc.gpsimd.dma_start(t_, x_d[:, c * 4096:(c + 1) * 4096])
        v3 = t_.rearrange("p (a t) -> p a t", a=8)
        xT_tiles[c] = [v3[:, kk, :] for kk in range(8)]

    emit_x_chunk(0)

    def load_wT(ap_d, prefix, engs):
        t_ = consts.tile([128, 8 * O], BF16, name=prefix)
        v3 = t_.rearrange("p (a o) -> p a o", a=8)
        src = ap_d.rearrange("(a p) o -> p a o", p=128)
        if len(engs) == 2:
            engs[0].dma_start(v3[:, 0:4, :], src[:, 0:4, :])
            engs[1].dma_start(v3[:, 4:8, :], src[:, 4:8, :])
        else:
            engs[0].dma_start(v3, src)
        return [v3[:, kk, :] for kk in range(8)]

    wq_sb = load_wT(wq_d, "wq", (nc.scalar, nc.sync))
    bq_sb = consts.tile([128, 2], f32, name="bq_sb")
    nc.scalar.dma_start(bq_sb, bq_d.rearrange("(c p) -> p c", p=128))
    wk_sb = load_wT(wk_d, "wk", (nc.sync,))
    wv_sb = load_wT(wv_d, "wv", (nc.gpsimd,))
    wo_sb = []
    for oc in range(2):
        t_ = consts.tile([128, K], BF16, name=f"wo{oc}")
        nc.gpsimd.dma_start(t_, wo_d[oc * 128:(oc + 1) * 128, :])
        wo_sb.append(t_)

    # ---------------- filler work items ----------------
    # Projections of chunk c+1 and the output projection of chunk c-1 are
    # emitted one item per attention r-iteration, so the priority order the
    # list scheduler sees matches the pipeline order we want: the attention
    # QK->exp->AV chain stays hot while every PE stall is filled with a
    # proj/outproj matmul group.  All of them share the "ps" psum ring so
    # the S^T double-buffer ring stays dedicated to attention.

    def proj_items(c):
        xT = xT_tiles.pop(c)

        def qk_item(w_sb, dest, oc, bias):
            def em():
                ps = pmm.tile([128, 512], f32, name="ps_qk", tag="ps")
                for kk in range(8):
                    nc.tensor.matmul(
                        ps, w_sb[kk][:, oc * 128:(oc + 1) * 128], xT[kk],
                        start=(kk == 0), stop=(kk == 7))
                if bias:
                    nc.vector.tensor_scalar_add(dest[c][oc], ps,
                                                bq_sb[:, oc:oc + 1])
                else:
                    nc.vector.tensor_copy(dest[c][oc], ps)
            return em

        def v_item(a):
            def em():
                ps = pmm.tile([128, O], f32, name="ps_v", tag="ps")
                for kk in range(8):
                    nc.tensor.matmul(
                        ps, xT[kk][:, a * 128:(a + 1) * 128], wv_sb[kk],
                        start=(kk == 0), stop=(kk == 7))
                dst = vv[c * 4 + a].rearrange("p (h x) -> p h x", x=S + 1)
                nc.vector.tensor_copy(
                    dst[:, :, 0:S], ps.rearrange("p (h x) -> p h x", x=S))
            return em

        items = []
        for w_sb, dest, bias in ((wq_sb, qT, True), (wk_sb, kT, False)):
            for oc in range(2):
                items.append(qk_item(w_sb, dest, oc, bias))
        for a in range(4):
            items.append(v_item(a))
        return items

    ytile = [0]

    def outproj_items(c, oT_c):
        def py_item(i, jc):
            def em():
                py = pmm.tile([128, 512], f32, name="py", tag="ps")
                for oc in range(2):
                    nc.tensor.matmul(
                        py, oT_c[oc][:, i * 128:(i + 1) * 128],
                        wo_sb[oc][:, jc * 512:(jc + 1) * 512],
                        start=(oc == 0), stop=(oc == 1))
                ys = ys_state.get(i)
                if ys is None:
                    ys = work.tile([128, K], f32, name="ystg", tag="ystg",
                                   bufs=4)
                    ys_state[i] = ys
                if jc == 0:
                    nc.vector.tensor_copy(ys[:, 0:512], py)
                else:
                    nc.scalar.copy(ys[:, 512:1024], py)
                    t0 = c * 512 + i * 128
                    eng = nc.sync if ytile[0] % 2 == 0 else nc.gpsimd
                    eng.dma_start(y_d[t0:t0 + 128, :], ys)
                    ytile[0] += 1
            return em

        ys_state = {}
        return [py_item(i, jc) for i in range(4) for jc in range(2)]

    # prologue: chunk-0 projections are the critical path, emit directly
    for em in proj_items(0):
        em()

    inv_scale = 1.0 / float(np.sqrt(K))
    fillers = []
    for c in range(4):
        if c < 3:
            emit_x_chunk(c + 1)
            fillers.extend(proj_items(c + 1))

        # ---------------- attention for q-chunk c ----------------
        oT_c = []
        for oc in range(2):
            po = [ppo.tile([S + 1, 512], f32, name=f"po{e}", tag=f"po{e}")
                  for e in range(2)]
            nr = 4 * (c + 1)  # causal: t_k tiles 0..4c+3
            for r in range(nr):
                m = r - 4 * c
                j0 = 128 * m if m > 0 else 0  # fully-masked cols skipped
                ps = pbig.tile([128, 1024], f32, name="ps_s", tag="pss")
                for e in range(2):
                    hb = e * 64
                    nc.tensor.matmul(
                        ps[:, e * 512 + j0:(e + 1) * 512],
                        kT[r // 4][oc][hb:hb + 64,
                                       (r % 4) * 128:(r % 4 + 1) * 128],
                        qT[c][oc][hb:hb + 64, j0:512],
                        start=True, stop=True)
                pt = work.tile([128, 1024], BF16, name="pt_exp", tag="ptl",
                               bufs=6)
                ps3 = ps.rearrange("p (e j) -> p e j", e=2)[:, :, j0:]
                pt3 = pt.rearrange("p (e j) -> p e j", e=2)[:, :, j0:]
                nc.scalar.activation(pt3, ps3, AF.Exp, scale=inv_scale)
                if m >= 0:
                    # only the first 128 trimmed columns are partial (bf16
                    # SBUF everywhere -> DVE 4x mode)
                    nc.vector.tensor_mul(
                        pt3[:, :, 0:128], pt3[:, :, 0:128], tm3)
                for e in range(2):
                    h = 2 * oc + e
                    nc.tensor.matmul(
                        po[e][:, j0:],
                        vv[r][:, h * (S + 1):(h + 1) * (S + 1)],
                        pt[:, e * 512 + j0:(e + 1) * 512],
                        start=(r == 0), stop=(r == nr - 1))
                if fillers:
                    fillers.pop(0)()

            # -------- evict O^T + denominators for this head pair --------
            oTc = work.tile([128, 512], BF16, name=f"oT{oc}", tag=f"oT{oc}",
                            bufs=2)
            nc.vector.tensor_copy(oTc[0:S, :], po[0][0:S, :])
            stg = work.tile([S, 512], BF16, name="stg", tag="stg", bufs=2)
            nc.vector.tensor_copy(stg, po[1][0:S, :])
            nc.sync.dma_start(oTc[S:128, :], stg)
            rs = work.tile([S + 1, 1024], F32R, name="rs", tag="rs", bufs=2)
            with nc.allow_low_precision(reason="f32r softmax denom recip"):
                nc.vector.reciprocal(rs[S:S + 1, 0:512], po[0][S:S + 1, :])
                nc.vector.reciprocal(rs[S:S + 1, 512:1024], po[1][S:S + 1, :])
            # rank-1 broadcast of the reciprocal rows across partitions
            # (PSUM matmul dst must start at partition 0, so broadcast each
            # head's row to all 128 partitions and multiply slice-aligned)
            prb = pbig.tile([128, 1024], f32, name="prb", tag="pss")
            nc.tensor.matmul(prb[:, 0:512], onesr[S:S + 1, :],
                             rs[S:S + 1, 0:512], start=True, stop=True)
            nc.tensor.matmul(prb[:, 512:1024], onesr[S:S + 1, :],
                             rs[S:S + 1, 512:1024], start=True, stop=True)
            nc.vector.tensor_mul(oTc[0:S, :], oTc[0:S, :], prb[0:S, 0:512])
            nc.vector.tensor_mul(oTc[S:128, :], oTc[S:128, :],
                                 prb[S:128, 512:1024])
            oT_c.append(oTc)

        fillers.extend(outproj_items(c, oT_c))

    for em in fillers:  # drain: outproj of chunk 3
        em()


def build_program(loop_n=0):
    nc = bacc.Bacc("TRN2", target_bir_lowering=False, debug=False,
                   num_devices=NCORES)
    d = {
        "x": nc.dram_tensor("xT", [128, 16384], BF16,
                            kind="ExternalInput").ap(),
        "wqT": nc.dram_tensor("wqT", [K, O], BF16, kind="ExternalInput").ap(),
        "wkT": nc.dram_tensor("wkT", [K, O], BF16, kind="ExternalInput").ap(),
        "wvT": nc.dram_tensor("wvT", [K, O], BF16, kind="ExternalInput").ap(),
        "woT": nc.dram_tensor("woT", [O, K], BF16, kind="ExternalInput").ap(),
        "bq": nc.dram_tensor("bq", [O], F32, kind="ExternalInput").ap(),
        "y": nc.dram_tensor("y", [T, K], F32, kind="ExternalOutput").ap(),
    }
    with tile.TileContext(nc) as tc:
        _build_body(nc, tc, d, loop_n=loop_n)
    nc.compile()
    return nc


def _get_program():
    if "nc" not in _CACHE:
        _CACHE["nc"] = build_program()
    return _CACHE["nc"]


def _bf16(a):
    import ml_dtypes
    return np.ascontiguousarray(np.asarray(a, np.float32).astype(
        ml_dtypes.bfloat16))


def _tile_x(xb):
    # [T, K] -> [p=128, c=4, a=8, t=512] -> [128, 16384]
    # dram[p, c*4096 + a*512 + t] = x[c*512 + t, a*128 + p]
    xt = np.asarray(xb, np.float32).T  # [K, T]
    xt = xt.reshape(8, 128, 4, 512).transpose(1, 2, 0, 3)
    return _bf16(xt.reshape(128, 16384))


def make_in_maps(x, Wq_w, Wk_w, Wv_w, Wo_w, Wq_b, Wk_b, Wv_b):
    in_maps = []
    for core in range(NCORES):
        b, hg = divmod(core, 4)
        sl = slice(hg * O, (hg + 1) * O)
        in_maps.append({
            "xT": _tile_x(np.asarray(x, np.float32)[b]),
            "wqT": _bf16(np.asarray(Wq_w)[sl, :].T),
            "wkT": _bf16(np.asarray(Wk_w)[sl, :].T),
            "wvT": _bf16(np.asarray(Wv_w)[sl, :].T),
            "woT": _bf16(np.asarray(Wo_w)[:, sl].T),
            "bq": np.ascontiguousarray(np.asarray(Wq_b)[sl], np.float32),
        })
    return in_maps


def _combine(results, Wo_w, Wv_b, Wo_b):
    # softmax rows sum to 1, so the V bias contributes Wo_w @ Wv_b exactly
    c0 = (np.asarray(Wo_w, np.float32) @ np.asarray(Wv_b, np.float32)
          + np.asarray(Wo_b, np.float32))
    y = np.empty((B, T, K), np.float32)
    for b in range(B):
        acc = results[b * 4]["y"].copy()
        for hg in range(1, 4):
            acc += results[b * 4 + hg]["y"]
        y[b] = acc + c0
    return y


def kernel(x, Wq_w, Wq_b, Wk_w, Wk_b, Wv_w, Wv_b, Wo_w, Wo_b):
    nc = _get_program()
    in_maps = make_in_maps(x, Wq_w, Wk_w, Wv_w, Wo_w, Wq_b, Wk_b, Wv_b)
    out = run_bass_kernel_spmd(nc, in_maps, list(range(NCORES)))
    return _combine(out.results, Wo_w, Wv_b, Wo_b)
